# revision 28
# baseline (speedup 1.0000x reference)
"""GRU-D Bass kernel for Trainium2, data-parallel over batch on 8 NeuronCores.

Math (reference reduction):
  M is binary {0,1}, so the GRU-D input decay collapses:
    x_hat  = m*x + (1-m)*xm
    g      = exp(-gamma*(1-m))         -> g=1 where m=1; where m=0, x_hat=xm so
    x_tilde= g*x_hat + (1-g)*xm        ->   x_tilde = m*x + (1-m)*xm exactly.
  With U = m*x:
    inp @ W.T + b = U @ W1.T + m @ (W3 - W1*xm).T + [xm @ (W1+W2).T + b]
  where W = [W1 W2 W3] column blocks. The r gate is unused by the reference.
  So per gate it is a K=512 GEMM over rows (b,t), plus a constant bias.
  z, h_til do not depend on h: compute them for ALL timesteps as one GEMM,
  then run the affine scan h = (1-z)*h + z*h_til with tensor_tensor_scan
  along the time (free) axis, keeping only the final column per sequence.
  Output: sigmoid(h_T @ Wout.T + bout).
"""

import numpy as np
import ml_dtypes

B, T, D, H = 512, 256, 256, 1024
NCORES = 8
PART = 128
KC = 4                      # contraction chunks (2D/128)
HC = 8                      # H chunks (H/128)
BLOCK_ROWS = 1024           # rows per pipeline block (= 4 sequences of T)
BL = B // NCORES            # sequences per core
ROWS = BL * T               # GEMM rows per core

_BF16 = ml_dtypes.bfloat16

_cache = {}


def _scan_hc(nc, Alu, zbig, hbig, hl, sl, seq0, spb, T_, Act=None):
    """Per-hc gating + scans (in-place over the hc slices of z/h)."""
    nc.vector.tensor_tensor(out=hbig[:, sl], in0=zbig[:, sl],
                            in1=hbig[:, sl], op=Alu.mult)
    if Act is not None:
        nc.scalar.activation(out=zbig[:, sl], in_=zbig[:, sl],
                             func=Act.Identity, scale=-1.0, bias=1.0)
    else:
        nc.vector.tensor_scalar(zbig[:, sl], zbig[:, sl], -1.0, 1.0,
                                Alu.mult, Alu.add)
    base = sl.start
    for s in range(spb):
        ssl = slice(base + s * T_, base + (s + 1) * T_)
        nc.vector.tensor_tensor_scan(
            out=hl[:, seq0 + s:seq0 + s + 1].broadcast_to([128, T_]),
            data0=zbig[:, ssl], data1=hbig[:, ssl],
            initial=0.0, op0=Alu.mult, op1=Alu.add)


def _build_nc(rows):
    import concourse.mybir as mybir
    import concourse.tile as tile
    from concourse import bacc

    f32 = mybir.dt.float32
    bf16 = mybir.dt.bfloat16
    Alu = mybir.AluOpType
    Act = mybir.ActivationFunctionType

    nseq = rows // T
    if rows >= 4 * BLOCK_ROWS:
        # two small lead blocks so the PE starts ~20us sooner
        blocks = [512, 512] + [BLOCK_ROWS] * ((rows - 1024) // BLOCK_ROWS)
    else:
        blocks = [BLOCK_ROWS] * (rows // BLOCK_ROWS)
    assert sum(blocks) == rows

    nc = bacc.Bacc("TRN2", target_bir_lowering=False, debug=False,
                   num_devices=NCORES, num_swdge_queues=4)

    x_d = nc.dram_tensor("x", [rows, D], f32, kind="ExternalInput").ap()
    m_d = nc.dram_tensor("m", [rows, D], f32, kind="ExternalInput").ap()
    wz_d = nc.dram_tensor("wzp", [KC, PART, H], bf16, kind="ExternalInput").ap()
    wh_d = nc.dram_tensor("whp", [KC, PART, H], bf16, kind="ExternalInput").ap()
    cz_d = nc.dram_tensor("czT", [PART, HC], f32, kind="ExternalInput").ap()
    ch_d = nc.dram_tensor("chT", [PART, HC], f32, kind="ExternalInput").ap()
    wo_d = nc.dram_tensor("woT", [PART, HC], f32, kind="ExternalInput").ap()
    bo_d = nc.dram_tensor("bo", [1, 1], f32, kind="ExternalInput").ap()
    out_d = nc.dram_tensor("out", [1, nseq], f32, kind="ExternalOutput").ap()

    with tile.TileContext(nc) as tc:
        with (
            tc.tile_pool(name="consts", bufs=1) as consts,
            tc.tile_pool(name="xin", bufs=2) as xin_pool,
            tc.tile_pool(name="um", bufs=3) as um_pool,
            tc.tile_pool(name="umt", bufs=3) as umt_pool,
            tc.tile_pool(name="zs", bufs=3) as z_pool,
            tc.tile_pool(name="hs", bufs=3) as h_pool,
            tc.tile_pool(name="hlast", bufs=1) as hl_pool,
            tc.tile_pool(name="outp", bufs=1) as out_pool,
            tc.tile_pool(name="psum", bufs=3, space="PSUM") as psum_pool,
            tc.tile_pool(name="psumh", bufs=1, space="PSUM") as psum_head,
        ):
            wz_sb, wh_sb = [], []
            for k in range(KC):
                t1 = consts.tile([PART, H], bf16, tag=f"wz{k}", name=f"wz{k}")
                nc.scalar.dma_start(out=t1[:], in_=wz_d[k])
                wz_sb.append(t1)
                t2 = consts.tile([PART, H], bf16, tag=f"wh{k}", name=f"wh{k}")
                nc.scalar.dma_start(out=t2[:], in_=wh_d[k])
                wh_sb.append(t2)
            czT = consts.tile([PART, HC], f32, tag="czT", name="czT")
            nc.scalar.dma_start(out=czT[:], in_=cz_d)
            chT = consts.tile([PART, HC], f32, tag="chT", name="chT")
            nc.scalar.dma_start(out=chT[:], in_=ch_d)
            woT = consts.tile([PART, HC], f32, tag="woT", name="woT")
            nc.scalar.dma_start(out=woT[:], in_=wo_d)
            boT = consts.tile([1, 1], f32, tag="boT", name="boT")
            nc.scalar.dma_start(out=boT[:], in_=bo_d)

            hlast = [hl_pool.tile([PART, nseq], f32, tag=f"hl{i}", name=f"hl{i}")
                     for i in range(HC)]

            Wg = (wz_sb, wh_sb)
            Cg = (czT, chT)
            Fg = (Act.Sigmoid, Act.Tanh)

            r0 = 0
            for blk, brows in enumerate(blocks):
                spb = brows // T
                seq0 = r0 // T
                nchunk = brows // PART
                nhalf = brows // 512
                xs = x_d[r0:r0 + brows, :].rearrange(
                    "(c p) d -> p c d", p=PART)
                ms = m_d[r0:r0 + brows, :].rearrange(
                    "(c p) d -> p c d", p=PART)
                um3 = um_pool.tile([PART, nchunk, 2 * D], bf16, tag="um3", name=f"um3_{blk}")
                if blk < 2 and brows < BLOCK_ROWS:
                    # SWDGE cast-DMAs take ~25us to land the first block;
                    # use fast HWDGE fp32 loads + on-chip cast instead
                    xf = xin_pool.tile([PART, nchunk, D], f32, tag="xf32", name=f"xf32_{blk}")
                    mf = xin_pool.tile([PART, nchunk, D], f32, tag="mf32", name=f"mf32_{blk}")
                    nc.sync.dma_start(out=xf[:], in_=xs)
                    nc.sync.dma_start(out=mf[:], in_=ms)
                    nc.vector.tensor_copy(out=um3[:, :, D:2 * D], in_=mf[:])
                    nc.vector.tensor_tensor(out=um3[:, :, 0:D], in0=mf[:],
                                            in1=xf[:], op=Alu.mult)
                else:
                    xbf = xin_pool.tile([PART, nchunk, D], bf16, tag="xbf", name=f"xbf{blk}")
                    nc.gpsimd.dma_start(out=xbf[:], in_=xs)
                    nc.gpsimd.dma_start(out=um3[:, :, D:2 * D], in_=ms)
                    # U = m * x, into the first D columns of each chunk
                    nc.vector.tensor_tensor(out=um3[:, :, 0:D],
                                            in0=um3[:, :, D:2 * D],
                                            in1=xbf[:], op=Alu.mult)

                # One xbar transpose for the whole block:
                # out[d, g, r] = in_[r, g*128 + d], g = c*KC + k.
                umt = umt_pool.tile([PART, brows * KC], bf16, tag="umt",
                                    name=f"umt{blk}")
                nc.sync.dma_start_transpose(
                    out=umt[:].rearrange("p (g r) -> p g r", r=PART),
                    in_=um3[:].rearrange("p c q -> p (c q)"))
                # free index = c*(KC*PART) + k*PART + r
                u4 = umt[:].rearrange("p (c s) -> p c s", s=KC * PART)

                # last block: hc-outer/gate-inner + per-hc gating so the
                # scan tail overlaps the GEMM; wide ops elsewhere
                perhc = blk == len(blocks) - 1
                zbig = z_pool.tile([PART, HC * brows], bf16, tag="z", name=f"z{blk}")
                hbig = h_pool.tile([PART, HC * brows], bf16, tag="h", name=f"h{blk}")
                order = ([(hc, gate) for hc in range(HC) for gate in (0, 1)]
                         if perhc else
                         [(hc, gate) for gate in (0, 1) for hc in range(HC)])
                for hc, gate in order:
                    ps = psum_pool.tile([PART, brows], f32, tag="ps", name=f"ps{blk}_{gate}_{hc}")
                    for k in range(KC):
                        lhsT = Wg[gate][k][:, hc * PART:(hc + 1) * PART]
                        for nh in range(nhalf):
                            nc.tensor.matmul(
                                out=ps[:, nh * 512:(nh + 1) * 512],
                                lhsT=lhsT,
                                rhs=u4[:, 4 * nh:4 * nh + 4,
                                       k * PART:(k + 1) * PART],
                                start=(k == 0), stop=(k == KC - 1))
                    dst = zbig if gate == 0 else hbig
                    nc.scalar.activation(
                        out=dst[:, hc * brows:(hc + 1) * brows],
                        in_=ps[:], func=Fg[gate],
                        bias=Cg[gate][:, hc:hc + 1])
                    if perhc and gate == 1:
                        self_sl = slice(hc * brows, (hc + 1) * brows)
                        _scan_hc(nc, Alu, zbig, hbig, hlast[hc],
                                 self_sl, seq0, spb, T, Act=Act)

                if not perhc:
                    # b = z * h_til (in place over h_til), one wide op
                    nc.vector.tensor_tensor(out=hbig[:], in0=zbig[:],
                                            in1=hbig[:], op=Alu.mult)
                    # a = 1 - z in place, on the Scalar engine (DVE is
                    # the critical engine overall; ACT has slack)
                    nc.scalar.activation(out=zbig[:], in_=zbig[:],
                                         func=Act.Identity,
                                         scale=-1.0, bias=1.0)
                    for hc in range(HC):
                        base = hc * brows
                        for s in range(spb):
                            sl = slice(base + s * T, base + (s + 1) * T)
                            col = seq0 + s
                            nc.vector.tensor_tensor_scan(
                                out=hlast[hc][:, col:col + 1].broadcast_to(
                                    [PART, T]),
                                data0=zbig[:, sl], data1=hbig[:, sl],
                                initial=0.0, op0=Alu.mult, op1=Alu.add)
                r0 += brows

            hp = psum_head.tile([1, nseq], f32, tag="hp", name="hp")
            for hc in range(HC):
                nc.tensor.matmul(out=hp[:], lhsT=woT[:, hc:hc + 1],
                                 rhs=hlast[hc][:],
                                 start=(hc == 0), stop=(hc == HC - 1))
            outt = out_pool.tile([1, nseq], f32, tag="outt", name="outt")
            nc.scalar.activation(out=outt[:], in_=hp[:], func=Act.Sigmoid,
                                 bias=boT[0:1, 0:1])
            nc.sync.dma_start(out=out_d, in_=outt[:])

    nc.compile()
    return nc


def _prep_weights(input_means, Wz, bz, Wh, bh, Wout, bout):
    xm = np.asarray(input_means, np.float32)

    def gate(Wg, bg):
        W1 = np.asarray(Wg[:, :D], np.float32)
        W2 = np.asarray(Wg[:, D:2 * D], np.float32)
        W3 = np.asarray(Wg[:, 2 * D:], np.float32)
        Wp = np.concatenate([W1.T, (W3 - W1 * xm[None, :]).T], axis=0)
        Wp = np.ascontiguousarray(
            Wp.reshape(KC, PART, H).astype(_BF16))
        c = ((W1 + W2) @ xm + np.asarray(bg, np.float32)).astype(np.float32)
        cT = np.ascontiguousarray(c.reshape(HC, PART).T)
        return Wp, cT

    wzp, czT = gate(Wz, bz)
    whp, chT = gate(Wh, bh)
    woT = np.ascontiguousarray(
        np.asarray(Wout, np.float32).reshape(HC, PART).T)
    bo = np.asarray(bout, np.float32).reshape(1, 1)
    return dict(wzp=wzp, whp=whp, czT=czT, chT=chT, woT=woT, bo=bo)


def _get_nc(rows):
    if rows not in _cache:
        _cache[rows] = _build_nc(rows)
    return _cache[rows]


def _install_ntff_shim():
    """The agent image lacks antenv.axon_hooks; recreate it so
    run_bass_kernel_spmd(trace=True) can capture NTFF profiles."""
    import sys
    import types
    try:
        import antenv.axon_hooks  # noqa: F401
        return
    except ImportError:
        pass
    mod = types.ModuleType("antenv.axon_hooks")
    mod._hook = None
    mod.set_axon_ntff_profile_hook = lambda h: setattr(mod, "_hook", h)
    mod.get_axon_ntff_profile_hook = lambda: mod._hook
    sys.modules["antenv.axon_hooks"] = mod
    from trn_agent_boot.trn_boot import _ntff_profile_via_ctypes
    mod.set_axon_ntff_profile_hook(
        _ntff_profile_via_ctypes("/opt/axon/libaxon_pjrt.so"))
    # avoid network artifact uploads in this container
    import concourse.bass_utils as bu
    bu.upload_artifacts = lambda tmpdir: "local://" + str(tmpdir)


def run(X, M, input_means, gamma_x, Wz, bz, Wr, br, Wh, bh, Wout, bout,
        trace=False, rows=ROWS, n_cores=NCORES):
    """Run the Bass kernel. Returns (out [n_cores*rows//T], BassKernelResults)."""
    from concourse.bass_utils import run_bass_kernel_spmd
    if trace:
        _install_ntff_shim()

    nc = _get_nc(rows)
    wmap = _prep_weights(input_means, Wz, bz, Wh, bh, Wout, bout)
    X = np.asarray(X, np.float32)
    M = np.asarray(M, np.float32)
    nseq_core = rows // T
    in_maps = []
    for c in range(n_cores):
        s0 = c * nseq_core
        xc = np.ascontiguousarray(
            X[s0:s0 + nseq_core].reshape(rows, D))
        mc = np.ascontiguousarray(
            M[s0:s0 + nseq_core].reshape(rows, D))
        in_maps.append({"x": xc, "m": mc, **wmap})
    res = run_bass_kernel_spmd(nc, in_maps, list(range(n_cores)), trace=trace)
    out = np.concatenate(
        [res.results[c]["out"].reshape(nseq_core) for c in range(n_cores)])
    return out.astype(np.float32), res


def kernel(X, M, input_means, gamma_x, Wz, bz, Wr, br, Wh, bh, Wout, bout):
    out, _ = run(X, M, input_means, gamma_x, Wz, bz, Wr, br, Wh, bh,
                 Wout, bout)
    return out


# revision 29
# speedup vs baseline: 1.4194x; 1.4194x over previous
"""GRU-D Bass kernel for Trainium2, data-parallel over batch on 8 NeuronCores.

Math (reference reduction):
  M is binary {0,1}, so the GRU-D input decay collapses:
    x_hat  = m*x + (1-m)*xm
    g      = exp(-gamma*(1-m))         -> g=1 where m=1; where m=0, x_hat=xm so
    x_tilde= g*x_hat + (1-g)*xm        ->   x_tilde = m*x + (1-m)*xm exactly.
  With U = m*x:
    inp @ W.T + b = U @ W1.T + m @ (W3 - W1*xm).T + [xm @ (W1+W2).T + b]
  where W = [W1 W2 W3] column blocks. The r gate is unused by the reference.
  So per gate it is a K=512 GEMM over rows (b,t), plus a constant bias.
  z, h_til do not depend on h: compute them for ALL timesteps as one GEMM,
  then run the affine scan h = (1-z)*h + z*h_til with tensor_tensor_scan
  along the time (free) axis, keeping only the final column per sequence.
  Output: sigmoid(h_T @ Wout.T + bout).
"""

import numpy as np
import ml_dtypes

B, T, D, H = 512, 256, 256, 1024
NCORES = 8
PART = 128
KC = 4                      # contraction chunks (2D/128)
HC = 8                      # H chunks (H/128)
BLOCK_ROWS = 1024           # rows per pipeline block (= 4 sequences of T)
BL = B // NCORES            # sequences per core
ROWS = BL * T               # GEMM rows per core

_BF16 = ml_dtypes.bfloat16

_cache = {}


def _scan_hc(nc, Alu, zbig, hbig, hl, sl, seq0, spb, T_):
    """Per-hc gating + scans (in-place over the hc slices of z/h)."""
    nc.vector.tensor_tensor(out=hbig[:, sl], in0=zbig[:, sl],
                            in1=hbig[:, sl], op=Alu.mult)
    nc.vector.tensor_scalar(zbig[:, sl], zbig[:, sl], -1.0, 1.0,
                            Alu.mult, Alu.add)
    base = sl.start
    for s in range(spb):
        ssl = slice(base + s * T_, base + (s + 1) * T_)
        nc.vector.tensor_tensor_scan(
            out=hl[:, seq0 + s:seq0 + s + 1].broadcast_to([128, T_]),
            data0=zbig[:, ssl], data1=hbig[:, ssl],
            initial=0.0, op0=Alu.mult, op1=Alu.add)


def _build_nc(rows):
    import concourse.mybir as mybir
    import concourse.tile as tile
    from concourse import bacc

    f32 = mybir.dt.float32
    bf16 = mybir.dt.bfloat16
    Alu = mybir.AluOpType
    Act = mybir.ActivationFunctionType

    nseq = rows // T
    if rows >= 4 * BLOCK_ROWS:
        # two small lead blocks so the PE starts ~20us sooner
        blocks = [512, 512] + [BLOCK_ROWS] * ((rows - 1024) // BLOCK_ROWS)
    else:
        blocks = [BLOCK_ROWS] * (rows // BLOCK_ROWS)
    assert sum(blocks) == rows

    nc = bacc.Bacc("TRN2", target_bir_lowering=False, debug=False,
                   num_devices=NCORES, num_swdge_queues=4)

    x_d = nc.dram_tensor("x", [rows, D], f32, kind="ExternalInput").ap()
    m_d = nc.dram_tensor("m", [rows, D], f32, kind="ExternalInput").ap()
    wz_d = nc.dram_tensor("wzp", [KC, PART, H], bf16, kind="ExternalInput").ap()
    wh_d = nc.dram_tensor("whp", [KC, PART, H], bf16, kind="ExternalInput").ap()
    cz_d = nc.dram_tensor("czT", [PART, HC], f32, kind="ExternalInput").ap()
    ch_d = nc.dram_tensor("chT", [PART, HC], f32, kind="ExternalInput").ap()
    wo_d = nc.dram_tensor("woT", [PART, HC], f32, kind="ExternalInput").ap()
    bo_d = nc.dram_tensor("bo", [1, 1], f32, kind="ExternalInput").ap()
    out_d = nc.dram_tensor("out", [1, nseq], f32, kind="ExternalOutput").ap()

    with tile.TileContext(nc) as tc:
        with (
            tc.tile_pool(name="consts", bufs=1) as consts,
            tc.tile_pool(name="xin", bufs=2) as xin_pool,
            tc.tile_pool(name="um", bufs=3) as um_pool,
            tc.tile_pool(name="umt", bufs=3) as umt_pool,
            tc.tile_pool(name="zs", bufs=3) as z_pool,
            tc.tile_pool(name="hs", bufs=3) as h_pool,
            tc.tile_pool(name="hlast", bufs=1) as hl_pool,
            tc.tile_pool(name="outp", bufs=1) as out_pool,
            tc.tile_pool(name="psum", bufs=3, space="PSUM") as psum_pool,
            tc.tile_pool(name="psumh", bufs=1, space="PSUM") as psum_head,
        ):
            wz_sb, wh_sb = [], []
            for k in range(KC):
                t1 = consts.tile([PART, H], bf16, tag=f"wz{k}", name=f"wz{k}")
                nc.scalar.dma_start(out=t1[:], in_=wz_d[k])
                wz_sb.append(t1)
                t2 = consts.tile([PART, H], bf16, tag=f"wh{k}", name=f"wh{k}")
                nc.scalar.dma_start(out=t2[:], in_=wh_d[k])
                wh_sb.append(t2)
            czT = consts.tile([PART, HC], f32, tag="czT", name="czT")
            nc.scalar.dma_start(out=czT[:], in_=cz_d)
            chT = consts.tile([PART, HC], f32, tag="chT", name="chT")
            nc.scalar.dma_start(out=chT[:], in_=ch_d)
            woT = consts.tile([PART, HC], f32, tag="woT", name="woT")
            nc.scalar.dma_start(out=woT[:], in_=wo_d)
            boT = consts.tile([1, 1], f32, tag="boT", name="boT")
            nc.scalar.dma_start(out=boT[:], in_=bo_d)

            hlast = [hl_pool.tile([PART, nseq], f32, tag=f"hl{i}", name=f"hl{i}")
                     for i in range(HC)]

            Wg = (wz_sb, wh_sb)
            Cg = (czT, chT)
            Fg = (Act.Sigmoid, Act.Tanh)

            r0 = 0
            for blk, brows in enumerate(blocks):
                spb = brows // T
                seq0 = r0 // T
                nchunk = brows // PART
                nhalf = brows // 512
                xs = x_d[r0:r0 + brows, :].rearrange(
                    "(c p) d -> p c d", p=PART)
                ms = m_d[r0:r0 + brows, :].rearrange(
                    "(c p) d -> p c d", p=PART)
                um3 = um_pool.tile([PART, nchunk, 2 * D], bf16, tag="um3", name=f"um3_{blk}")
                if blk < 2 and brows < BLOCK_ROWS:
                    # SWDGE cast-DMAs take ~25us to land the first block;
                    # use fast HWDGE fp32 loads + on-chip cast instead
                    xf = xin_pool.tile([PART, nchunk, D], f32, tag="xf32", name=f"xf32_{blk}")
                    mf = xin_pool.tile([PART, nchunk, D], f32, tag="mf32", name=f"mf32_{blk}")
                    nc.sync.dma_start(out=xf[:], in_=xs)
                    nc.sync.dma_start(out=mf[:], in_=ms)
                    nc.vector.tensor_copy(out=um3[:, :, D:2 * D], in_=mf[:])
                    nc.vector.tensor_tensor(out=um3[:, :, 0:D], in0=mf[:],
                                            in1=xf[:], op=Alu.mult)
                else:
                    xbf = xin_pool.tile([PART, nchunk, D], bf16, tag="xbf", name=f"xbf{blk}")
                    nc.gpsimd.dma_start(out=xbf[:], in_=xs)
                    nc.gpsimd.dma_start(out=um3[:, :, D:2 * D], in_=ms)
                    # U = m * x, into the first D columns of each chunk
                    nc.vector.tensor_tensor(out=um3[:, :, 0:D],
                                            in0=um3[:, :, D:2 * D],
                                            in1=xbf[:], op=Alu.mult)

                # One xbar transpose for the whole block:
                # out[d, g, r] = in_[r, g*128 + d], g = c*KC + k.
                umt = umt_pool.tile([PART, brows * KC], bf16, tag="umt",
                                    name=f"umt{blk}")
                nc.sync.dma_start_transpose(
                    out=umt[:].rearrange("p (g r) -> p g r", r=PART),
                    in_=um3[:].rearrange("p c q -> p (c q)"))
                # free index = c*(KC*PART) + k*PART + r
                u4 = umt[:].rearrange("p (c s) -> p c s", s=KC * PART)

                # last block: hc-outer/gate-inner + per-hc gating so the
                # scan tail overlaps the GEMM; wide ops elsewhere
                perhc = blk == len(blocks) - 1
                zbig = z_pool.tile([PART, HC * brows], bf16, tag="z", name=f"z{blk}")
                hbig = h_pool.tile([PART, HC * brows], bf16, tag="h", name=f"h{blk}")
                order = ([(hc, gate) for hc in range(HC) for gate in (0, 1)]
                         if perhc else
                         [(hc, gate) for gate in (0, 1) for hc in range(HC)])
                for hc, gate in order:
                    ps = psum_pool.tile([PART, brows], f32, tag="ps", name=f"ps{blk}_{gate}_{hc}")
                    for k in range(KC):
                        lhsT = Wg[gate][k][:, hc * PART:(hc + 1) * PART]
                        for nh in range(nhalf):
                            nc.tensor.matmul(
                                out=ps[:, nh * 512:(nh + 1) * 512],
                                lhsT=lhsT,
                                rhs=u4[:, 4 * nh:4 * nh + 4,
                                       k * PART:(k + 1) * PART],
                                start=(k == 0), stop=(k == KC - 1))
                    dst = zbig if gate == 0 else hbig
                    nc.scalar.activation(
                        out=dst[:, hc * brows:(hc + 1) * brows],
                        in_=ps[:], func=Fg[gate],
                        bias=Cg[gate][:, hc:hc + 1])
                    if perhc and gate == 1:
                        self_sl = slice(hc * brows, (hc + 1) * brows)
                        _scan_hc(nc, Alu, zbig, hbig, hlast[hc],
                                 self_sl, seq0, spb, T)

                if not perhc:
                    # b = z * h_til (in place over h_til), one wide op
                    nc.vector.tensor_tensor(out=hbig[:], in0=zbig[:],
                                            in1=hbig[:], op=Alu.mult)
                    # a = 1 - z (in place over z), one wide op
                    nc.vector.tensor_scalar(zbig[:], zbig[:], -1.0, 1.0,
                                            Alu.mult, Alu.add)
                    for hc in range(HC):
                        base = hc * brows
                        for s in range(spb):
                            sl = slice(base + s * T, base + (s + 1) * T)
                            col = seq0 + s
                            nc.vector.tensor_tensor_scan(
                                out=hlast[hc][:, col:col + 1].broadcast_to(
                                    [PART, T]),
                                data0=zbig[:, sl], data1=hbig[:, sl],
                                initial=0.0, op0=Alu.mult, op1=Alu.add)
                r0 += brows

            hp = psum_head.tile([1, nseq], f32, tag="hp", name="hp")
            for hc in range(HC):
                nc.tensor.matmul(out=hp[:], lhsT=woT[:, hc:hc + 1],
                                 rhs=hlast[hc][:],
                                 start=(hc == 0), stop=(hc == HC - 1))
            outt = out_pool.tile([1, nseq], f32, tag="outt", name="outt")
            nc.scalar.activation(out=outt[:], in_=hp[:], func=Act.Sigmoid,
                                 bias=boT[0:1, 0:1])
            nc.sync.dma_start(out=out_d, in_=outt[:])

    nc.compile()
    return nc


def _prep_weights(input_means, Wz, bz, Wh, bh, Wout, bout):
    xm = np.asarray(input_means, np.float32)

    def gate(Wg, bg):
        W1 = np.asarray(Wg[:, :D], np.float32)
        W2 = np.asarray(Wg[:, D:2 * D], np.float32)
        W3 = np.asarray(Wg[:, 2 * D:], np.float32)
        Wp = np.concatenate([W1.T, (W3 - W1 * xm[None, :]).T], axis=0)
        Wp = np.ascontiguousarray(
            Wp.reshape(KC, PART, H).astype(_BF16))
        c = ((W1 + W2) @ xm + np.asarray(bg, np.float32)).astype(np.float32)
        cT = np.ascontiguousarray(c.reshape(HC, PART).T)
        return Wp, cT

    wzp, czT = gate(Wz, bz)
    whp, chT = gate(Wh, bh)
    woT = np.ascontiguousarray(
        np.asarray(Wout, np.float32).reshape(HC, PART).T)
    bo = np.asarray(bout, np.float32).reshape(1, 1)
    return dict(wzp=wzp, whp=whp, czT=czT, chT=chT, woT=woT, bo=bo)


def _get_nc(rows):
    if rows not in _cache:
        _cache[rows] = _build_nc(rows)
    return _cache[rows]


def _install_ntff_shim():
    """The agent image lacks antenv.axon_hooks; recreate it so
    run_bass_kernel_spmd(trace=True) can capture NTFF profiles."""
    import sys
    import types
    try:
        import antenv.axon_hooks  # noqa: F401
        return
    except ImportError:
        pass
    mod = types.ModuleType("antenv.axon_hooks")
    mod._hook = None
    mod.set_axon_ntff_profile_hook = lambda h: setattr(mod, "_hook", h)
    mod.get_axon_ntff_profile_hook = lambda: mod._hook
    sys.modules["antenv.axon_hooks"] = mod
    from trn_agent_boot.trn_boot import _ntff_profile_via_ctypes
    mod.set_axon_ntff_profile_hook(
        _ntff_profile_via_ctypes("/opt/axon/libaxon_pjrt.so"))
    # avoid network artifact uploads in this container
    import concourse.bass_utils as bu
    bu.upload_artifacts = lambda tmpdir: "local://" + str(tmpdir)


def run(X, M, input_means, gamma_x, Wz, bz, Wr, br, Wh, bh, Wout, bout,
        trace=False, rows=ROWS, n_cores=NCORES):
    """Run the Bass kernel. Returns (out [n_cores*rows//T], BassKernelResults)."""
    from concourse.bass_utils import run_bass_kernel_spmd
    if trace:
        _install_ntff_shim()

    nc = _get_nc(rows)
    wmap = _prep_weights(input_means, Wz, bz, Wh, bh, Wout, bout)
    X = np.asarray(X, np.float32)
    M = np.asarray(M, np.float32)
    nseq_core = rows // T
    in_maps = []
    for c in range(n_cores):
        s0 = c * nseq_core
        xc = np.ascontiguousarray(
            X[s0:s0 + nseq_core].reshape(rows, D))
        mc = np.ascontiguousarray(
            M[s0:s0 + nseq_core].reshape(rows, D))
        in_maps.append({"x": xc, "m": mc, **wmap})
    res = run_bass_kernel_spmd(nc, in_maps, list(range(n_cores)), trace=trace)
    out = np.concatenate(
        [res.results[c]["out"].reshape(nseq_core) for c in range(n_cores)])
    return out.astype(np.float32), res


def kernel(X, M, input_means, gamma_x, Wz, bz, Wr, br, Wh, bh, Wout, bout):
    out, _ = run(X, M, input_means, gamma_x, Wz, bz, Wr, br, Wh, bh,
                 Wout, bout)
    return out


# revision 30
# speedup vs baseline: 1.4266x; 1.0051x over previous
"""GRU-D Bass kernel for Trainium2, data-parallel over batch on 8 NeuronCores.

Math (reference reduction):
  M is binary {0,1}, so the GRU-D input decay collapses:
    x_hat  = m*x + (1-m)*xm
    g      = exp(-gamma*(1-m))         -> g=1 where m=1; where m=0, x_hat=xm so
    x_tilde= g*x_hat + (1-g)*xm        ->   x_tilde = m*x + (1-m)*xm exactly.
  With U = m*x:
    inp @ W.T + b = U @ W1.T + m @ (W3 - W1*xm).T + [xm @ (W1+W2).T + b]
  where W = [W1 W2 W3] column blocks. The r gate is unused by the reference.
  So per gate it is a K=512 GEMM over rows (b,t), plus a constant bias.
  z, h_til do not depend on h: compute them for ALL timesteps as one GEMM,
  then run the affine scan h = (1-z)*h + z*h_til with tensor_tensor_scan
  along the time (free) axis, keeping only the final column per sequence.
  Output: sigmoid(h_T @ Wout.T + bout).
"""

import numpy as np
import ml_dtypes

B, T, D, H = 512, 256, 256, 1024
NCORES = 8
PART = 128
KC = 4                      # contraction chunks (2D/128)
HC = 8                      # H chunks (H/128)
BLOCK_ROWS = 1024           # rows per pipeline block (= 4 sequences of T)
BL = B // NCORES            # sequences per core
ROWS = BL * T               # GEMM rows per core

_BF16 = ml_dtypes.bfloat16

_cache = {}


def _scan_hc(nc, Alu, zbig, hbig, hl, sl, seq0, spb, T_):
    """Per-hc gating + scans (in-place over the hc slices of z/h)."""
    nc.vector.tensor_tensor(out=hbig[:, sl], in0=zbig[:, sl],
                            in1=hbig[:, sl], op=Alu.mult)
    nc.vector.tensor_scalar(zbig[:, sl], zbig[:, sl], -1.0, 1.0,
                            Alu.mult, Alu.add)
    base = sl.start
    for s in range(spb):
        ssl = slice(base + s * T_, base + (s + 1) * T_)
        nc.vector.tensor_tensor_scan(
            out=hl[:, seq0 + s:seq0 + s + 1].broadcast_to([128, T_]),
            data0=zbig[:, ssl], data1=hbig[:, ssl],
            initial=0.0, op0=Alu.mult, op1=Alu.add)


def _build_nc(rows):
    import concourse.mybir as mybir
    import concourse.tile as tile
    from concourse import bacc

    f32 = mybir.dt.float32
    bf16 = mybir.dt.bfloat16
    Alu = mybir.AluOpType
    Act = mybir.ActivationFunctionType

    nseq = rows // T
    if rows >= 4 * BLOCK_ROWS:
        # two small lead blocks so the PE starts ~20us sooner
        blocks = [512, 512] + [BLOCK_ROWS] * ((rows - 1024) // BLOCK_ROWS)
    else:
        blocks = [BLOCK_ROWS] * (rows // BLOCK_ROWS)
    assert sum(blocks) == rows

    nc = bacc.Bacc("TRN2", target_bir_lowering=False, debug=False,
                   num_devices=NCORES, num_swdge_queues=4)

    x_d = nc.dram_tensor("x", [rows, D], f32, kind="ExternalInput").ap()
    m_d = nc.dram_tensor("m", [rows, D], f32, kind="ExternalInput").ap()
    wz_d = nc.dram_tensor("wzp", [KC, PART, H], bf16, kind="ExternalInput").ap()
    wh_d = nc.dram_tensor("whp", [KC, PART, H], bf16, kind="ExternalInput").ap()
    cz_d = nc.dram_tensor("czT", [PART, HC], f32, kind="ExternalInput").ap()
    ch_d = nc.dram_tensor("chT", [PART, HC], f32, kind="ExternalInput").ap()
    wo_d = nc.dram_tensor("woT", [PART, HC], f32, kind="ExternalInput").ap()
    bo_d = nc.dram_tensor("bo", [1, 1], f32, kind="ExternalInput").ap()
    out_d = nc.dram_tensor("out", [1, nseq], f32, kind="ExternalOutput").ap()

    with tile.TileContext(nc) as tc:
        with (
            tc.tile_pool(name="consts", bufs=1) as consts,
            tc.tile_pool(name="xin", bufs=2) as xin_pool,
            tc.tile_pool(name="um", bufs=3) as um_pool,
            tc.tile_pool(name="umt", bufs=3) as umt_pool,
            tc.tile_pool(name="zs", bufs=3) as z_pool,
            tc.tile_pool(name="hs", bufs=3) as h_pool,
            tc.tile_pool(name="hlast", bufs=1) as hl_pool,
            tc.tile_pool(name="outp", bufs=1) as out_pool,
            tc.tile_pool(name="psum", bufs=4, space="PSUM") as psum_pool,
        ):
            wz_sb, wh_sb = [], []
            for k in range(KC):
                t1 = consts.tile([PART, H], bf16, tag=f"wz{k}", name=f"wz{k}")
                nc.scalar.dma_start(out=t1[:], in_=wz_d[k])
                wz_sb.append(t1)
                t2 = consts.tile([PART, H], bf16, tag=f"wh{k}", name=f"wh{k}")
                nc.scalar.dma_start(out=t2[:], in_=wh_d[k])
                wh_sb.append(t2)
            czT = consts.tile([PART, HC], f32, tag="czT", name="czT")
            nc.scalar.dma_start(out=czT[:], in_=cz_d)
            chT = consts.tile([PART, HC], f32, tag="chT", name="chT")
            nc.scalar.dma_start(out=chT[:], in_=ch_d)
            woT = consts.tile([PART, HC], f32, tag="woT", name="woT")
            nc.scalar.dma_start(out=woT[:], in_=wo_d)
            boT = consts.tile([1, 1], f32, tag="boT", name="boT")
            nc.scalar.dma_start(out=boT[:], in_=bo_d)

            hlast = [hl_pool.tile([PART, nseq], f32, tag=f"hl{i}", name=f"hl{i}")
                     for i in range(HC)]

            Wg = (wz_sb, wh_sb)
            Cg = (czT, chT)
            Fg = (Act.Sigmoid, Act.Tanh)

            r0 = 0
            for blk, brows in enumerate(blocks):
                spb = brows // T
                seq0 = r0 // T
                nchunk = brows // PART
                nhalf = brows // 512
                xs = x_d[r0:r0 + brows, :].rearrange(
                    "(c p) d -> p c d", p=PART)
                ms = m_d[r0:r0 + brows, :].rearrange(
                    "(c p) d -> p c d", p=PART)
                um3 = um_pool.tile([PART, nchunk, 2 * D], bf16, tag="um3", name=f"um3_{blk}")
                if blk < 2 and brows < BLOCK_ROWS:
                    # SWDGE cast-DMAs take ~25us to land the first block;
                    # use fast HWDGE fp32 loads + on-chip cast instead
                    xf = xin_pool.tile([PART, nchunk, D], f32, tag="xf32", name=f"xf32_{blk}")
                    mf = xin_pool.tile([PART, nchunk, D], f32, tag="mf32", name=f"mf32_{blk}")
                    nc.sync.dma_start(out=xf[:], in_=xs)
                    nc.sync.dma_start(out=mf[:], in_=ms)
                    nc.vector.tensor_copy(out=um3[:, :, D:2 * D], in_=mf[:])
                    nc.vector.tensor_tensor(out=um3[:, :, 0:D], in0=mf[:],
                                            in1=xf[:], op=Alu.mult)
                else:
                    xbf = xin_pool.tile([PART, nchunk, D], bf16, tag="xbf", name=f"xbf{blk}")
                    nc.gpsimd.dma_start(out=xbf[:], in_=xs)
                    nc.gpsimd.dma_start(out=um3[:, :, D:2 * D], in_=ms)
                    # U = m * x, into the first D columns of each chunk
                    nc.vector.tensor_tensor(out=um3[:, :, 0:D],
                                            in0=um3[:, :, D:2 * D],
                                            in1=xbf[:], op=Alu.mult)

                # One xbar transpose for the whole block:
                # out[d, g, r] = in_[r, g*128 + d], g = c*KC + k.
                umt = umt_pool.tile([PART, brows * KC], bf16, tag="umt",
                                    name=f"umt{blk}")
                nc.sync.dma_start_transpose(
                    out=umt[:].rearrange("p (g r) -> p g r", r=PART),
                    in_=um3[:].rearrange("p c q -> p (c q)"))
                # free index = c*(KC*PART) + k*PART + r
                u4 = umt[:].rearrange("p (c s) -> p c s", s=KC * PART)

                # last block: hc-outer/gate-inner + per-hc gating so the
                # scan tail overlaps the GEMM; wide ops elsewhere
                perhc = blk == len(blocks) - 1
                zbig = z_pool.tile([PART, HC * brows], bf16, tag="z", name=f"z{blk}")
                hbig = h_pool.tile([PART, HC * brows], bf16, tag="h", name=f"h{blk}")
                order = ([(hc, gate) for hc in range(HC) for gate in (0, 1)]
                         if perhc else
                         [(hc, gate) for gate in (0, 1) for hc in range(HC)])
                for hc, gate in order:
                    ps = psum_pool.tile([PART, brows], f32, tag="ps", name=f"ps{blk}_{gate}_{hc}")
                    for k in range(KC):
                        lhsT = Wg[gate][k][:, hc * PART:(hc + 1) * PART]
                        for nh in range(nhalf):
                            nc.tensor.matmul(
                                out=ps[:, nh * 512:(nh + 1) * 512],
                                lhsT=lhsT,
                                rhs=u4[:, 4 * nh:4 * nh + 4,
                                       k * PART:(k + 1) * PART],
                                start=(k == 0), stop=(k == KC - 1))
                    dst = zbig if gate == 0 else hbig
                    nc.scalar.activation(
                        out=dst[:, hc * brows:(hc + 1) * brows],
                        in_=ps[:], func=Fg[gate],
                        bias=Cg[gate][:, hc:hc + 1])
                    if perhc and gate == 1:
                        self_sl = slice(hc * brows, (hc + 1) * brows)
                        _scan_hc(nc, Alu, zbig, hbig, hlast[hc],
                                 self_sl, seq0, spb, T)

                if not perhc:
                    # b = z * h_til (in place over h_til), one wide op
                    nc.vector.tensor_tensor(out=hbig[:], in0=zbig[:],
                                            in1=hbig[:], op=Alu.mult)
                    # a = 1 - z (in place over z), one wide op
                    nc.vector.tensor_scalar(zbig[:], zbig[:], -1.0, 1.0,
                                            Alu.mult, Alu.add)
                    for hc in range(HC):
                        base = hc * brows
                        for s in range(spb):
                            sl = slice(base + s * T, base + (s + 1) * T)
                            col = seq0 + s
                            nc.vector.tensor_tensor_scan(
                                out=hlast[hc][:, col:col + 1].broadcast_to(
                                    [PART, T]),
                                data0=zbig[:, sl], data1=hbig[:, sl],
                                initial=0.0, op0=Alu.mult, op1=Alu.add)
                r0 += brows

            hpt = psum_pool.tile([PART, BLOCK_ROWS], f32, tag="ps", name="hp")
            hp = hpt[0:1, 0:nseq]
            for hc in range(HC):
                nc.tensor.matmul(out=hp, lhsT=woT[:, hc:hc + 1],
                                 rhs=hlast[hc][:],
                                 start=(hc == 0), stop=(hc == HC - 1))
            outt = out_pool.tile([1, nseq], f32, tag="outt", name="outt")
            nc.scalar.activation(out=outt[:], in_=hp, func=Act.Sigmoid,
                                 bias=boT[0:1, 0:1])
            nc.sync.dma_start(out=out_d, in_=outt[:])

    nc.compile()
    return nc


def _prep_weights(input_means, Wz, bz, Wh, bh, Wout, bout):
    xm = np.asarray(input_means, np.float32)

    def gate(Wg, bg):
        W1 = np.asarray(Wg[:, :D], np.float32)
        W2 = np.asarray(Wg[:, D:2 * D], np.float32)
        W3 = np.asarray(Wg[:, 2 * D:], np.float32)
        Wp = np.concatenate([W1.T, (W3 - W1 * xm[None, :]).T], axis=0)
        Wp = np.ascontiguousarray(
            Wp.reshape(KC, PART, H).astype(_BF16))
        c = ((W1 + W2) @ xm + np.asarray(bg, np.float32)).astype(np.float32)
        cT = np.ascontiguousarray(c.reshape(HC, PART).T)
        return Wp, cT

    wzp, czT = gate(Wz, bz)
    whp, chT = gate(Wh, bh)
    woT = np.ascontiguousarray(
        np.asarray(Wout, np.float32).reshape(HC, PART).T)
    bo = np.asarray(bout, np.float32).reshape(1, 1)
    return dict(wzp=wzp, whp=whp, czT=czT, chT=chT, woT=woT, bo=bo)


def _get_nc(rows):
    if rows not in _cache:
        _cache[rows] = _build_nc(rows)
    return _cache[rows]


def _install_ntff_shim():
    """The agent image lacks antenv.axon_hooks; recreate it so
    run_bass_kernel_spmd(trace=True) can capture NTFF profiles."""
    import sys
    import types
    try:
        import antenv.axon_hooks  # noqa: F401
        return
    except ImportError:
        pass
    mod = types.ModuleType("antenv.axon_hooks")
    mod._hook = None
    mod.set_axon_ntff_profile_hook = lambda h: setattr(mod, "_hook", h)
    mod.get_axon_ntff_profile_hook = lambda: mod._hook
    sys.modules["antenv.axon_hooks"] = mod
    from trn_agent_boot.trn_boot import _ntff_profile_via_ctypes
    mod.set_axon_ntff_profile_hook(
        _ntff_profile_via_ctypes("/opt/axon/libaxon_pjrt.so"))
    # avoid network artifact uploads in this container
    import concourse.bass_utils as bu
    bu.upload_artifacts = lambda tmpdir: "local://" + str(tmpdir)


def run(X, M, input_means, gamma_x, Wz, bz, Wr, br, Wh, bh, Wout, bout,
        trace=False, rows=ROWS, n_cores=NCORES):
    """Run the Bass kernel. Returns (out [n_cores*rows//T], BassKernelResults)."""
    from concourse.bass_utils import run_bass_kernel_spmd
    if trace:
        _install_ntff_shim()

    nc = _get_nc(rows)
    wmap = _prep_weights(input_means, Wz, bz, Wh, bh, Wout, bout)
    X = np.asarray(X, np.float32)
    M = np.asarray(M, np.float32)
    nseq_core = rows // T
    in_maps = []
    for c in range(n_cores):
        s0 = c * nseq_core
        xc = np.ascontiguousarray(
            X[s0:s0 + nseq_core].reshape(rows, D))
        mc = np.ascontiguousarray(
            M[s0:s0 + nseq_core].reshape(rows, D))
        in_maps.append({"x": xc, "m": mc, **wmap})
    res = run_bass_kernel_spmd(nc, in_maps, list(range(n_cores)), trace=trace)
    out = np.concatenate(
        [res.results[c]["out"].reshape(nseq_core) for c in range(n_cores)])
    return out.astype(np.float32), res


def kernel(X, M, input_means, gamma_x, Wz, bz, Wr, br, Wh, bh, Wout, bout):
    out, _ = run(X, M, input_means, gamma_x, Wz, bz, Wr, br, Wh, bh,
                 Wout, bout)
    return out
